# revision 1
# baseline (speedup 1.0000x reference)
"""Trainium2 Bass kernel for nn_Attention_fusion (sparse_attention fusion block).

Self-contained: takes FULL inputs (B=8 batches), shards batch across 8
NeuronCores (pure data parallel), runs a single fused Bass/Tile kernel per
core, and gathers the full [8,128,128,128] output. BatchNorm batch statistics
are combined across cores with an on-device AllReduce of per-channel moment
partial sums.
"""
import sys

sys.path.insert(0, "/opt/trn_rl_repo")

import numpy as np

import concourse.bass as bass
import concourse.tile as tile
from concourse import mybir
from concourse.bass_utils import run_bass_kernel_spmd

B, C, H, W = 8, 128, 128, 128
N = H * W
HEADS, HD = 8, 16
EPS_BN = 1e-5
EPS_LN = 1e-5
NCORES = 8
TS = 512                    # free-dim tile size
NT = N // TS                # 32 tiles
PW = W + 2                  # padded row stride (130)
PN = PW * (H + 2)           # padded plane (130*130)

F32 = mybir.dt.float32
F16 = mybir.dt.float16
AX = mybir.AxisListType
ALU = mybir.AluOpType
ACT = mybir.ActivationFunctionType


# ----------------------------------------------------------------------------
# walrus workaround: this container's walrus rejects instructions with more
# than one sync wait command; split extra waits onto standalone EventSemaphore
# instructions on the same engine (program order preserves semantics).
def _split_sync_waits(nc, maxw=1):
    cnt = 0
    for f in nc.m.functions:
        for b in f.blocks:
            insts = b.instructions
            out = []
            changed = False
            for inst in insts:
                si = inst.sync_info
                waits = list(si.on_wait) if si and si.on_wait else []
                if len(waits) > maxw:
                    keep = waits[-maxw:] if maxw > 0 else []
                    extra = waits[: len(waits) - maxw]
                    for wz in extra:
                        es = mybir.InstEventSemaphore(
                            name=f"WSPLIT-{cnt}", ins=[], outs=[]
                        )
                        cnt += 1
                        es.engine = inst.engine
                        es.sync_info = mybir.SyncInfo(on_wait=[wz], on_update=[])
                        out.append(es)
                    inst.sync_info = mybir.SyncInfo(
                        on_wait=keep,
                        on_update=list(si.on_update) if si.on_update else [],
                    )
                    changed = True
                out.append(inst)
            if changed:
                del insts[:]
                insts.extend(out)
    return cnt


# ----------------------------------------------------------------------------
# Host-side weight preparation (identical for every core). All folds:
#  - LN gamma/beta folded into downstream conv weights / biases
#  - mean-centering matrix Cm = I - 11^T/128 folded into the ep weights
#  - attention scale folded into Wk
def _prep_weights(p):
    f32 = lambda a: np.ascontiguousarray(a, np.float32)
    f16 = lambda a: np.ascontiguousarray(a, np.float16)
    w = {}
    J = np.full((C, C), 1.0 / C, np.float64)
    Cm = np.eye(C) - J

    # channel-weights MLP:  y[4C] -> relu(W1 y + b1) -> sigmoid(W2 h + b2)
    W1 = p["cw_w1"].astype(np.float64)      # [512,512]
    W2 = p["cw_w2"].astype(np.float64)      # [256,512]
    # lhsT chunks: W1T[kc] = W1[:, kc*128:(kc+1)*128].T  -> [128, 4, 512]
    w["cw_w1T"] = f16(np.stack([W1[:, k * 128:(k + 1) * 128].T for k in range(4)], 1))
    w["cw_b1"] = f32(p["cw_b1"].reshape(4, 128).T)             # [128,4]
    w["cw_w2T"] = f16(np.stack([W2[:, k * 128:(k + 1) * 128].T for k in range(4)], 1))  # [128,4,256]
    w["cw_b2"] = f32(p["cw_b2"].reshape(2, 128).T)             # [128,2]

    # spatial weight
    w["sw_w1T"] = f16(p["sw_w1"].T)                            # [128,128]
    w["sw_b1"] = f32(p["sw_b1"].reshape(C, 1))
    w["sw_w2T_rep"] = f16(np.repeat(p["sw_w2"].reshape(C, 1), C, 1))  # [128,128] replicated cols
    w["sw_b2"] = f32(np.full((C, 1), p["sw_b2"][0], np.float32))

    # cross-path projections
    w["cp3T"] = f16(p["cp3_w"].T)
    w["cp3_b"] = f32(p["cp3_b"].reshape(C, 1))
    w["cp4T"] = f16(p["cp4_w"].T)
    w["cp4_b"] = f32(p["cp4_b"].reshape(C, 1))

    # attention: ctxpreT_i = scale * (Wv_i G_i Wk_i^T)  [(h,e),(h,d)]
    scale = HD ** -0.5
    for i, kvw in ((1, p["kv1_w"]), (2, p["kv2_w"])):
        kvw = kvw.astype(np.float64)
        Wk, Wv = kvw[:C], kvw[C:]
        w[f"WkT{i}s"] = f16(Wk.T * scale)                      # [c,(h,d)] scaled
        w[f"WvT{i}"] = f16(Wv.T)                               # [c,(h,e)]

    # ep weights (with Cm centering fold): zc = Cm x + (Cm Ws) s1 + (Cm Wa) a1 + Cm b
    for i, (epw, epb) in ((1, (p["ep1_w"], p["ep1_b"])), (2, (p["ep2_w"], p["ep2_b"]))):
        epw = epw.astype(np.float64)
        Ws, Wa = epw[:, :C], epw[:, C:]
        w[f"WsT{i}c"] = f16((Cm @ Ws).T)                       # [128,128]
        w[f"WaT{i}c"] = f16((Cm @ Wa).T)                       # rhs for device lhsT_a build
        w[f"epb{i}c"] = f32((Cm @ epb.astype(np.float64)).reshape(C, 1))  # [C,1] bias
    w["Cm"] = f16(Cm)                                          # lhsT for the x term (sym)
    w["Jdiv"] = f16(J)                                         # lhsT for var bcast (sym)

    # merge consumers with LN gamma/beta fold
    g1, b1 = p["ln1_g"].astype(np.float64), p["ln1_b"].astype(np.float64)
    g2, b2 = p["ln2_g"].astype(np.float64), p["ln2_b"].astype(np.float64)
    Dres = p["ce_res_w"].astype(np.float64)                    # [128,256]
    Dce1 = p["ce1_w"].astype(np.float64)                       # [128,256]
    w["resT1"] = f16((Dres[:, :C] * g1).T)
    w["resT2"] = f16((Dres[:, C:] * g2).T)
    w["res_bias"] = f32((Dres[:, :C] @ b1 + Dres[:, C:] @ b2).reshape(C, 1))
    w["ce1T1"] = f16((Dce1[:, :C] * g1).T)
    w["ce1T2"] = f16((Dce1[:, C:] * g2).T)
    w["ce1_bias"] = f32((Dce1[:, :C] @ b1 + Dce1[:, C:] @ b2
                         + p["ce1_b"].astype(np.float64)).reshape(C, 1))

    # depthwise 3x3 as 9 diagonal matrices [9,128,128] (tap order dy-major)
    dwk = p["ce_dw_w"].reshape(C, 3, 3).astype(np.float64)
    dwd = np.zeros((9, C, C), np.float64)
    for t in range(9):
        dy, dx = divmod(t, 3)
        np.fill_diagonal(dwd[t], dwk[:, dy, dx])
    w["dw_diag"] = f16(np.transpose(dwd, (1, 0, 2)))           # [C, 9, C]
    w["dw_b"] = f32(p["ce_dw_b"].reshape(C, 1))

    w["ce2T"] = f16(p["ce2_w"].T)
    w["ce2_b"] = f32(p["ce2_b"].reshape(C, 1))

    # additive attention mask: 0 on diagonal head blocks, -30000 elsewhere
    m = np.full((C, C), -30000.0, np.float32)
    for hh in range(HEADS):
        m[hh * HD:(hh + 1) * HD, hh * HD:(hh + 1) * HD] = 0.0
    w["attn_mask"] = f32(m)

    # BN params
    w["bn1_g"] = f32(p["ce_bn1_g"].reshape(C, 1))
    w["bn1_b"] = f32(p["ce_bn1_b"].reshape(C, 1))
    w["bn2_g"] = f32(p["ce_bn2_g"].reshape(C, 1))
    w["bn2_b"] = f32(p["ce_bn2_b"].reshape(C, 1))
    return w


_WSPECS = None


def _weight_specs(w):
    # name -> (shape, dtype) for dram tensor declaration
    return {k: (list(v.shape), mybir.dt.from_np(v.dtype)) for k, v in w.items()}


# ----------------------------------------------------------------------------
def _build(wspecs, ncores=NCORES, debug=False, split=True):
    nc = bass.Bass("TRN2", target_bir_lowering=False, debug=False,
                   num_devices=ncores)
    x1d = nc.dram_tensor("x1", [C, N], F32, kind="ExternalInput").ap()
    x2d = nc.dram_tensor("x2", [C, N], F32, kind="ExternalInput").ap()
    outd = nc.dram_tensor("out", [C, N], F32, kind="ExternalOutput").ap()
    wd = {k: nc.dram_tensor(k, shp, dt, kind="ExternalInput").ap()
          for k, (shp, dt) in wspecs.items()}
    dbg = {}
    if debug:
        for name, shp in [("zc1", [C, N]), ("u1", [C, N]), ("c1p", [C, N]),
                          ("res", [C, N]), ("e", [C, N]), ("bd2", [C, C]),
                          ("stats", [C, 8])]:
            dbg[name] = nc.dram_tensor("dbg_" + name, shp, F32,
                                       kind="ExternalOutput").ap()

    with tile.TileContext(nc) as tc:
        _body(nc, tc, x1d, x2d, outd, wd, dbg, ncores)
    if split:
        _split_sync_waits(nc)
    return nc


def _act_rsqrt(nc, out, in_, bias_ap):
    """out = rsqrt(in_ + bias) via the reciprocal_sqrt ACT table (bass's
    activation() refuses Rsqrt; accuracy is adequate for the fp16 path and
    verified against the reference end-to-end)."""
    eng = nc.scalar
    ins = [eng.lower_ap(in_), eng.lower_ap(bias_ap),
           mybir.ImmediateValue(dtype=F32, value=1.0),
           mybir.ImmediateValue(dtype=F32, value=0.0)]
    return eng.add_instruction(
        mybir.InstActivation(
            name=nc.get_next_instruction_name(),
            func=ACT.Rsqrt,
            ins=ins,
            outs=[eng.lower_ap(out)],
        ))


def _body(nc, tc, x1d, x2d, outd, wd, dbg, ncores):
    from contextlib import ExitStack
    ctx = ExitStack()
    with ctx:
        wpool = ctx.enter_context(tc.tile_pool(name="w", bufs=1))
        small = ctx.enter_context(tc.tile_pool(name="small", bufs=1))
        dram = ctx.enter_context(tc.tile_pool(name="dram", bufs=1, space="DRAM"))
        # phase-scoped pools (LIFO)
        pxs_cm = tc.tile_pool(name="pxs", bufs=1)
        pxs = pxs_cm.__enter__()
        pcp_cm = tc.tile_pool(name="pcp", bufs=1)
        pcp = pcp_cm.__enter__()
        ptr1_cm = tc.tile_pool(name="ptr1", bufs=2)
        trans = ptr1_cm.__enter__()
        ps1_cm = tc.tile_pool(name="ps1", bufs=2, space="PSUM")
        ps1 = ps1_cm.__enter__()

        # ---- load weights ------------------------------------------------
        wt = {}
        for k in wd:
            shp = list(wd[k].shape)
            if len(shp) == 3:
                flat = [shp[0], shp[1] * shp[2]]
                wt[k] = wpool.tile(flat, wd[k].dtype, tag=k, name="w_" + k)
                nc.sync.dma_start(wt[k][:], wd[k].rearrange("a b c -> a (b c)"))
            else:
                wt[k] = wpool.tile(shp, wd[k].dtype, tag=k, name="w_" + k)
                nc.sync.dma_start(wt[k][:], wd[k][:])

        # ---- input load (cast fp32 -> fp16), 4 chunks per tensor ---------
        xs = [pxs.tile([C, N], F16, tag="x1s", name="x1s"),
              pxs.tile([C, N], F16, tag="x2s", name="x2s")]
        NCH = 4
        CH = N // NCH
        for c_ in range(NCH):
            for i, xd in ((0, x1d), (1, x2d)):
                nc.gpsimd.dma_start(xs[i][:, c_ * CH:(c_ + 1) * CH],
                                    xd[:, c_ * CH:(c_ + 1) * CH])

        c1ps = pcp.tile([C, N], F16, tag="c1ps", name="c1ps")
        swbs2 = [pcp.tile([C, N], F16, tag="swb1", name="swb1"),
                 pcp.tile([C, N], F16, tag="swb2", name="swb2")]
        ones_row = small.tile([1, TS], F16, tag="ones_row")
        nc.vector.memset(ones_row[:], 1.0)
        eps_ln = small.tile([C, 1], F32, tag="eps_ln")
        nc.vector.memset(eps_ln[:], EPS_LN)
        eps_bn = small.tile([C, 1], F32, tag="eps_bn")
        nc.vector.memset(eps_bn[:], EPS_BN)

        # ---- phase A: spatial-weight chain (independent of channel MLP) --
        swT1, swB1 = wt["sw_w1T"], wt["sw_b1"]
        swT2r, swB2 = wt["sw_w2T_rep"], wt["sw_b2"]
        for i in range(2):
            for t in range(NT):
                sl = bass.ts(t, TS)
                ph = ps1.tile([C, TS], F32, tag="swh")
                nc.tensor.matmul(ph[:], swT1[:], xs[i][:, sl], start=True, stop=True)
                h_t = trans.tile([C, TS], F16, tag="h_t", bufs=2)
                nc.scalar.activation(h_t[:], ph[:], ACT.Relu, bias=swB1[:])
                pl = ps1.tile([C, TS], F32, tag="swl")
                nc.tensor.matmul(pl[:], swT2r[:], h_t[:], start=True, stop=True)
                nc.scalar.activation(swbs2[i][:, sl], pl[:],
                                     ACT.Sigmoid, bias=swB2[:])

        # ---- phase 0: channel stats + MLP --------------------------------
        stat_y = small.tile([C, 4], F32, tag="stat_y")       # avg1 avg2 mx1 mx2
        for i in range(2):
            nc.scalar.activation(c1ps[:], xs[i][:], ACT.Copy,
                                 accum_out=stat_y[:, i:i + 1])
            nc.vector.tensor_reduce(
                out=stat_y[:, 2 + i:3 + i], in_=xs[i][:], axis=AX.X, op=ALU.max)
        y16 = small.tile([C, 4], F16, tag="y16")
        nc.vector.tensor_scalar(out=y16[:, 0:2], in0=stat_y[:, 0:2],
                                scalar1=1.0 / N, scalar2=None, op0=ALU.mult)
        nc.vector.tensor_copy(y16[:, 2:4], stat_y[:, 2:4])

        h16 = small.tile([C, 4], F16, tag="h16")
        for oc in range(4):
            ph = ps1.tile([C, 1], F32, tag="ps_a")
            for kc in range(4):
                nc.tensor.matmul(
                    ph[:], wt["cw_w1T"][:, kc * 512 + oc * 128: kc * 512 + (oc + 1) * 128],
                    y16[:, kc:kc + 1], start=(kc == 0), stop=(kc == 3))
            nc.scalar.activation(h16[:, oc:oc + 1], ph[:], ACT.Relu,
                                 bias=wt["cw_b1"][:, oc:oc + 1])
        cw = small.tile([C, 2], F32, tag="cw")
        for oc in range(2):
            ph = ps1.tile([C, 1], F32, tag="ps_a")
            for kc in range(4):
                nc.tensor.matmul(
                    ph[:], wt["cw_w2T"][:, kc * 256 + oc * 128: kc * 256 + (oc + 1) * 128],
                    h16[:, kc:kc + 1], start=(kc == 0), stop=(kc == 3))
            nc.scalar.activation(cw[:, oc:oc + 1], ph[:], ACT.Sigmoid,
                                 bias=wt["cw_b2"][:, oc:oc + 1])
        # fold cw into cp lhsT: cpTc[i] = cpT[i] * cw_i (per-partition)
        cpTc = []
        for i in range(2):
            cc = small.tile([C, C], F16, tag=f"cpTc_{i}")
            nc.vector.tensor_scalar(out=cc[:], in0=wt[("cp3T", "cp4T")[i]][:],
                                    scalar1=cw[:, i:i + 1], scalar2=None,
                                    op0=ALU.mult)
            cpTc.append(cc)

        # ---- phase 1: cp chain + Gram ------------------------------------
        gp = ps1.tile([C, 2 * C], F32, tag="gram", bufs=1, name="gram")
        cpB = [wt["cp3_b"], wt["cp4_b"]]
        for i in range(2):
            for t in range(NT):
                sl = bass.ts(t, TS)
                pc = ps1.tile([C, TS], F32, tag="ps_a")
                nc.tensor.matmul(pc[:], cpTc[i][:], xs[i][:, sl], start=True, stop=True)
                if i == 0:
                    cdst = c1ps[:, sl]
                else:
                    c2t = trans.tile([C, TS], F16, tag="c2t", bufs=2)
                    cdst = c2t[:]
                nc.scalar.activation(cdst, pc[:], ACT.Relu, bias=cpB[i][:])
                pt = ps1.tile([C, TS], F16, tag="p1ct", bufs=1)
                for j in range(4):
                    nc.tensor.transpose(
                        pt[:, j * 128:(j + 1) * 128],
                        cdst[:, j * 128:(j + 1) * 128] if i == 1 else
                        c1ps[:, t * TS + j * 128: t * TS + (j + 1) * 128],
                        wt["ident"][:])
                ctt = trans.tile([C, TS], F16, tag="ctt")
                nc.vector.tensor_copy(ctt[:], pt[:])
                for j in range(4):
                    nc.tensor.matmul(gp[:, i * C:(i + 1) * C],
                                     ctt[:, j * 128:(j + 1) * 128],
                                     ctt[:, j * 128:(j + 1) * 128],
                                     start=(t == 0 and j == 0),
                                     stop=(t == NT - 1 and j == 3))

        # ---- phase 1b: attention context smalls --------------------------
        bdp = []
        for i in range(2):
            g16 = small.tile([C, C], F16, tag=f"g16_{i}")
            nc.vector.tensor_copy(g16[:], gp[:, i * C:(i + 1) * C])
            pm = ps1.tile([C, C], F32, tag="ps_a")
            nc.tensor.matmul(pm[:], g16[:], wt[f"WkT{i+1}s"][:], start=True, stop=True)
            m16 = small.tile([C, C], F16, tag=f"m16_{i}")
            nc.vector.tensor_copy(m16[:], pm[:])
            pc2 = ps1.tile([C, C], F32, tag="ps_a")
            nc.tensor.matmul(pc2[:], wt[f"WvT{i+1}"][:], m16[:], start=True, stop=True)
            cm_t = small.tile([C, C], F32, tag=f"cm_{i}")
            nc.vector.tensor_tensor(cm_t[:], pc2[:], wt["attn_mask"][:], op=ALU.add)
            negmx = small.tile([C, 1], F32, tag=f"negmx_{i}")
            nc.vector.tensor_reduce(out=negmx[:], in_=cm_t[:], axis=AX.X,
                                    op=ALU.max, negate=True)
            ex = small.tile([C, C], F32, tag=f"ex_{i}")
            nc.scalar.activation(ex[:], cm_t[:], ACT.Exp, bias=negmx[:])
            sm = small.tile([C, 1], F32, tag=f"sm_{i}")
            nc.vector.tensor_reduce(out=sm[:], in_=ex[:], axis=AX.X, op=ALU.add)
            rs = small.tile([C, 1], F32, tag=f"rs_{i}")
            nc.vector.reciprocal(rs[:], sm[:])
            bd = small.tile([C, C], F16, tag=f"bd_{i}")
            nc.vector.tensor_scalar(out=bd[:], in0=ex[:], scalar1=rs[:],
                                    scalar2=None, op0=ALU.mult)
            bdp.append(bd)
        if dbg:
            bddump = small.tile([C, C], F32, tag="bddump")
            nc.vector.tensor_copy(bddump[:], bdp[1][:])
            nc.sync.dma_start(dbg["bd2"][:], bddump[:])

        lhsTa = []
        for i in range(2):
            pa = ps1.tile([C, C], F32, tag="ps_a")
            nc.tensor.matmul(pa[:], bdp[1 - i][:], wt[f"WaT{i+1}c"][:],
                             start=True, stop=True)
            a16 = small.tile([C, C], F16, tag=f"a16_{i}")
            nc.vector.tensor_copy(a16[:], pa[:])
            lhsTa.append(a16)

        # ---- phase 2: ep + LN, u overwrites x store ----------------------
        ps1_cm.__exit__(None, None, None)
        ps2_cm = tc.tile_pool(name="ps2", bufs=3, space="PSUM")
        ps2 = ps2_cm.__enter__()
        WsTc = [wt["WsT1c"], wt["WsT2c"]]
        epbc = [wt["epb1c"], wt["epb2c"]]
        for i in range(2):
            for t in range(NT):
                sl = bass.ts(t, TS)
                s1_t = trans.tile([C, TS], F16, tag="s1_t", bufs=2)
                nc.vector.tensor_tensor(s1_t[:], xs[i][:, sl],
                                        swbs2[i][:, sl], op=ALU.mult)
                if i == 0:
                    crhs = c1ps[:, sl]
                else:
                    # recompute c2' (c2 store traded for swb2 SBUF space)
                    pc = ps2.tile([C, TS], F32, tag="cpr", bufs=2)
                    nc.tensor.matmul(pc[:], cpTc[1][:], xs[1][:, sl],
                                     start=True, stop=True)
                    c2t = trans.tile([C, TS], F16, tag="c2rt", bufs=2)
                    nc.scalar.activation(c2t[:], pc[:], ACT.Relu, bias=cpB[1][:])
                    crhs = c2t[:]
                pz = ps2.tile([C, TS], F32, tag="ep")
                nc.tensor.matmul(pz[:], WsTc[i][:], s1_t[:], start=True, stop=False)
                nc.tensor.matmul(pz[:], lhsTa[i][:], crhs, start=False, stop=False)
                nc.tensor.matmul(pz[:], wt["Cm"][:], xs[i][:, sl], start=False, stop=True)
                zc_t = trans.tile([C, TS], F16, tag="zc_t", bufs=2)
                nc.vector.tensor_scalar(out=zc_t[:], in0=pz[:], scalar1=epbc[i][:],
                                        scalar2=None, op0=ALU.add)
                if dbg and i == 0:
                    nc.gpsimd.dma_start(dbg["zc1"][:, sl], zc_t[:])
                zc2_t = trans.tile([C, TS], F16, tag="zc2_t", bufs=2)
                nc.vector.tensor_tensor(zc2_t[:], zc_t[:], zc_t[:], op=ALU.mult)
                pv = ps2.tile([C, TS], F32, tag="var", bufs=2)
                nc.tensor.matmul(pv[:], wt["Jdiv"][:], zc2_t[:], start=True, stop=True)
                rstd_t = trans.tile([C, TS], F16, tag="rstd_t", bufs=2)
                _act_rsqrt(nc, rstd_t[:], pv[:], eps_ln[:])
                nc.vector.tensor_tensor(xs[i][:, sl], zc_t[:], rstd_t[:], op=ALU.mult)
                if dbg and i == 0:
                    nc.gpsimd.dma_start(dbg["u1"][:, sl], xs[i][:, sl])
        if dbg:
            nc.gpsimd.dma_start(dbg["c1p"][:], c1ps[:])

        # ---- phase 3: merge consumers ------------------------------------
        ps2_cm.__exit__(None, None, None)
        ptr1_cm.__exit__(None, None, None)
        pcp_cm.__exit__(None, None, None)
        ps3_cm = tc.tile_pool(name="ps3", bufs=2, space="PSUM")
        ps3 = ps3_cm.__enter__()
        pconv_cm = tc.tile_pool(name="pconv", bufs=1)
        pconv = pconv_cm.__enter__()
        ptr2_cm = tc.tile_pool(name="ptr2", bufs=2)
        trans = ptr2_cm.__enter__()
        res_s = pconv.tile([C, N], F16, tag="res_s")
        e1p = pconv.tile([C, PN], F16, tag="e1p")
        nc.vector.memset(e1p[:], 0.0)   # border stays zero
        prt = small.tile([C, NT], F32, tag="prt")
        prt2 = small.tile([C, NT], F32, tag="prt2")
        pet = small.tile([C, NT], F32, tag="pet")
        pet2 = small.tile([C, NT], F32, tag="pet2")
        pert = small.tile([C, NT], F32, tag="pert")
        zero1 = small.tile([C, 1], F32, tag="zero1")
        nc.vector.memset(zero1[:], 0.0)
        for t in range(NT):
            sl = bass.ts(t, TS)
            pr = ps3.tile([C, TS], F32, tag="resp")
            nc.tensor.matmul(pr[:], wt["resT1"][:], xs[0][:, sl], start=True, stop=False)
            nc.tensor.matmul(pr[:], wt["resT2"][:], xs[1][:, sl], start=False, stop=True)
            nc.scalar.activation(res_s[:, sl], pr[:], ACT.Identity,
                                 bias=wt["res_bias"][:],
                                 accum_out=prt[:, t:t + 1])
            pe1 = ps3.tile([C, TS], F32, tag="e1ps")
            nc.tensor.matmul(pe1[:], wt["ce1T1"][:], xs[0][:, sl], start=True, stop=False)
            nc.tensor.matmul(pe1[:], wt["ce1T2"][:], xs[1][:, sl], start=False, stop=True)
            h0 = t * 4
            e1v = e1p[:].rearrange("p (h w) -> p h w", w=PW)
            nc.scalar.activation(
                e1v[:, h0 + 1: h0 + 5, 1: 1 + W], pe1[:].rearrange("p (a b) -> p a b", b=W),
                ACT.Identity, bias=wt["ce1_bias"][:])
            junk_t = trans.tile([C, TS], F16, tag="junk_t")
            nc.scalar.activation(junk_t[:], res_s[:, sl], ACT.Square,
                                 accum_out=prt2[:, t:t + 1])

        # ---- phase 4: depthwise conv + ce2 + e moments --------------------
        e_s = pxs.tile([C, N], F16, tag="x1s", name="e_s")
        e1v = e1p[:].rearrange("p (h w) -> p h w", w=PW)
        for t in range(NT):
            sl = bass.ts(t, TS)
            h0 = t * 4
            pdw = ps3.tile([C, TS], F32, tag="dw")
            for tap in range(9):
                dy, dx = divmod(tap, 3)
                rhs = e1v[:, h0 + dy: h0 + dy + 4, dx: dx + W]
                nc.tensor.matmul(pdw[:], wt["dw_diag"][:, tap * C:(tap + 1) * C],
                                 rhs, start=(tap == 0), stop=(tap == 8))
            e2_t = trans.tile([C, TS], F16, tag="e2_t")
            nc.scalar.activation(e2_t[:], pdw[:], ACT.Relu, bias=wt["dw_b"][:])
            pce = ps3.tile([C, TS], F32, tag="ce2p")
            nc.tensor.matmul(pce[:], wt["ce2T"][:], e2_t[:], start=True, stop=True)
            nc.scalar.activation(e_s[:, sl], pce[:], ACT.Identity,
                                 bias=wt["ce2_b"][:], accum_out=pet[:, t:t + 1])
            junk2_t = trans.tile([C, TS], F16, tag="junk_t")
            nc.scalar.activation(junk2_t[:], e_s[:, sl], ACT.Square,
                                 accum_out=pet2[:, t:t + 1])
            er_t = trans.tile([C, TS], F16, tag="er_t")
            nc.vector.tensor_tensor(er_t[:], e_s[:, sl], res_s[:, sl], op=ALU.mult)
            nc.vector.tensor_reduce(out=pert[:, t:t + 1], in_=er_t[:], axis=AX.X,
                                    op=ALU.add)

        # ---- phase 5: cross-core moment allreduce + final affine ----------
        parts = small.tile([C, 8], F32, tag="parts")
        nc.vector.memset(parts[:], 0.0)
        nc.vector.tensor_reduce(out=parts[:, 0:1], in_=pet[:], axis=AX.X, op=ALU.add)
        nc.vector.tensor_reduce(out=parts[:, 1:2], in_=pet2[:], axis=AX.X, op=ALU.add)
        nc.vector.tensor_reduce(out=parts[:, 2:3], in_=prt[:], axis=AX.X, op=ALU.add)
        nc.vector.tensor_reduce(out=parts[:, 3:4], in_=prt2[:], axis=AX.X, op=ALU.add)
        nc.vector.tensor_reduce(out=parts[:, 4:5], in_=pert[:], axis=AX.X, op=ALU.add)
        cin = dram.tile([C, 8], F32)
        cout = dram.tile([C, 8], F32)
        nc.sync.dma_start(cin[:], parts[:])
        nc.gpsimd.collective_compute(
            "AllReduce", ALU.add,
            replica_groups=[list(range(ncores))],
            ins=[cin.opt()], outs=[cout.opt()])
        allr = small.tile([C, 8], F32, tag="allr")
        nc.sync.dma_start(allr[:], cout[:])

        # per-channel BN coefficient math (tiny [128,1] ops)
        NTOT = float(ncores * N)
        cf = small.tile([C, 12], F32, tag="cf")
        nc.vector.tensor_scalar(out=cf[:, 0:1], in0=allr[:, 0:1], scalar1=1.0 / NTOT,
                                scalar2=None, op0=ALU.mult)               # me
        nc.vector.tensor_scalar(out=cf[:, 1:2], in0=allr[:, 1:2], scalar1=1.0 / NTOT,
                                scalar2=None, op0=ALU.mult)               # E e2
        nc.vector.tensor_tensor(cf[:, 2:3], cf[:, 0:1], cf[:, 0:1], op=ALU.mult)
        nc.vector.tensor_tensor(cf[:, 2:3], cf[:, 1:2], cf[:, 2:3], op=ALU.subtract)  # ve
        tmp = small.tile([C, 4], F32, tag="cftmp")
        nc.scalar.activation(tmp[:, 0:1], cf[:, 2:3], ACT.Sqrt, bias=eps_bn[:])
        nc.vector.reciprocal(tmp[:, 1:2], tmp[:, 0:1])                     # rsq_e
        nc.vector.tensor_tensor(cf[:, 3:4], wt["bn1_g"][:], tmp[:, 1:2], op=ALU.mult)  # a1
        nc.vector.tensor_tensor(cf[:, 4:5], cf[:, 0:1], cf[:, 3:4], op=ALU.mult)
        nc.vector.tensor_tensor(cf[:, 4:5], wt["bn1_b"][:], cf[:, 4:5], op=ALU.subtract)  # b1f
        nc.vector.tensor_tensor(cf[:, 5:6], cf[:, 3:4], allr[:, 0:1], op=ALU.mult)
        nc.vector.tensor_tensor(cf[:, 5:6], allr[:, 2:3], cf[:, 5:6], op=ALU.add)
        nc.vector.tensor_scalar(out=cf[:, 5:6], in0=cf[:, 5:6], scalar1=1.0 / NTOT,
                                scalar2=None, op0=ALU.mult)
        nc.vector.tensor_tensor(cf[:, 5:6], cf[:, 5:6], cf[:, 4:5], op=ALU.add)   # ms
        nc.vector.tensor_tensor(tmp[:, 2:3], cf[:, 3:4], cf[:, 3:4], op=ALU.mult)  # a1^2
        nc.vector.tensor_tensor(tmp[:, 3:4], tmp[:, 2:3], allr[:, 1:2], op=ALU.mult)
        nc.vector.tensor_tensor(cf[:, 6:7], cf[:, 3:4], allr[:, 4:5], op=ALU.mult)
        nc.vector.tensor_scalar(out=cf[:, 6:7], in0=cf[:, 6:7], scalar1=2.0,
                                scalar2=None, op0=ALU.mult)
        nc.vector.tensor_tensor(cf[:, 6:7], cf[:, 6:7], tmp[:, 3:4], op=ALU.add)
        nc.vector.tensor_tensor(cf[:, 6:7], cf[:, 6:7], allr[:, 3:4], op=ALU.add)
        nc.vector.tensor_scalar(out=cf[:, 6:7], in0=cf[:, 6:7], scalar1=1.0 / NTOT,
                                scalar2=None, op0=ALU.mult)
        nc.vector.tensor_tensor(tmp[:, 2:3], cf[:, 5:6], cf[:, 4:5], op=ALU.subtract)
        nc.vector.tensor_tensor(tmp[:, 2:3], tmp[:, 2:3], cf[:, 4:5], op=ALU.mult)
        nc.vector.tensor_scalar(out=tmp[:, 2:3], in0=tmp[:, 2:3], scalar1=2.0,
                                scalar2=None, op0=ALU.mult)
        nc.vector.tensor_tensor(cf[:, 6:7], cf[:, 6:7], tmp[:, 2:3], op=ALU.add)
        nc.vector.tensor_tensor(tmp[:, 2:3], cf[:, 4:5], cf[:, 4:5], op=ALU.mult)
        nc.vector.tensor_tensor(cf[:, 6:7], cf[:, 6:7], tmp[:, 2:3], op=ALU.add)  # Es2
        nc.vector.tensor_tensor(tmp[:, 2:3], cf[:, 5:6], cf[:, 5:6], op=ALU.mult)
        nc.vector.tensor_tensor(cf[:, 7:8], cf[:, 6:7], tmp[:, 2:3], op=ALU.subtract)  # vs
        nc.scalar.activation(tmp[:, 0:1], cf[:, 7:8], ACT.Sqrt, bias=eps_bn[:])
        nc.vector.reciprocal(tmp[:, 1:2], tmp[:, 0:1])
        nc.vector.tensor_tensor(cf[:, 8:9], wt["bn2_g"][:], tmp[:, 1:2], op=ALU.mult)  # a2
        nc.vector.tensor_tensor(cf[:, 9:10], cf[:, 8:9], cf[:, 3:4], op=ALU.mult)      # ae
        # c0 = b2 - a2*(ms - b1f)
        nc.vector.tensor_tensor(tmp[:, 2:3], cf[:, 5:6], cf[:, 4:5], op=ALU.subtract)
        nc.vector.tensor_tensor(tmp[:, 2:3], tmp[:, 2:3], cf[:, 8:9], op=ALU.mult)
        nc.vector.tensor_tensor(cf[:, 10:11], wt["bn2_b"][:], tmp[:, 2:3],
                                op=ALU.subtract)

        if dbg:
            nc.sync.dma_start(dbg["stats"][:], allr[:])
            nc.gpsimd.dma_start(dbg["res"][:], res_s[:])
            nc.gpsimd.dma_start(dbg["e"][:], e_s[:])

        for t in range(NT):
            sl = bass.ts(t, TS)
            t1 = trans.tile([C, TS], F32, tag="fin1")
            nc.vector.tensor_scalar(out=t1[:], in0=e_s[:, sl], scalar1=cf[:, 9:10],
                                    scalar2=cf[:, 10:11], op0=ALU.mult, op1=ALU.add)
            t2 = trans.tile([C, TS], F32, tag="fin2")
            nc.vector.tensor_scalar(out=t2[:], in0=res_s[:, sl], scalar1=cf[:, 8:9],
                                    scalar2=None, op0=ALU.mult)
            out_t = trans.tile([C, TS], F32, tag="out_t")
            nc.vector.tensor_tensor(out_t[:], t1[:], t2[:], op=ALU.add)
            nc.sync.dma_start(outd[:, sl], out_t[:])
        ptr2_cm.__exit__(None, None, None)
        pconv_cm.__exit__(None, None, None)
        ps3_cm.__exit__(None, None, None)
        pxs_cm.__exit__(None, None, None)


_CACHE = {}


def _get_nc(wspecs_key, wspecs, ncores=NCORES, debug=False):
    key = (wspecs_key, ncores, debug)
    if key not in _CACHE:
        _CACHE[key] = _build(wspecs, ncores=ncores, debug=debug)
    return _CACHE[key]


def kernel(**inputs):
    w = _prep_weights(inputs)
    w["ident"] = np.eye(C, dtype=np.float16)
    wspecs = _weight_specs(w)
    nc = _get_nc("v1", wspecs)
    x1 = np.ascontiguousarray(inputs["x1"], np.float32).reshape(B, C, N)
    x2 = np.ascontiguousarray(inputs["x2"], np.float32).reshape(B, C, N)
    in_maps = [dict(w, x1=x1[b], x2=x2[b]) for b in range(B)]
    res = run_bass_kernel_spmd(nc, in_maps, list(range(NCORES)))
    out = np.stack([res.results[b]["out"].reshape(C, H, W) for b in range(B)])
    return out.astype(np.float32)


def _ensure_ntff_hook():
    """Synthesize antenv.axon_hooks (missing in this image) and register the
    ctypes NTFF profile hook against the axon PJRT .so."""
    import types
    import antenv
    if getattr(antenv, "axon_hooks", None) is not None:
        return
    mod = types.ModuleType("antenv.axon_hooks")
    mod._hook = None
    def set_axon_ntff_profile_hook(h):
        mod._hook = h
    def get_axon_ntff_profile_hook():
        return mod._hook
    mod.set_axon_ntff_profile_hook = set_axon_ntff_profile_hook
    mod.get_axon_ntff_profile_hook = get_axon_ntff_profile_hook
    sys.modules["antenv.axon_hooks"] = mod
    antenv.axon_hooks = mod
    try:
        sys.path.insert(0, "/root/.axon_site")
        from trn_agent_boot.trn_boot import _ntff_profile_via_ctypes
        hook = _ntff_profile_via_ctypes("/opt/axon/libaxon_pjrt.so")
        if hook is not None:
            mod._hook = hook
    except Exception as e:
        print(f"ntff hook setup failed: {e}")


def timed_run(**inputs):
    """Run once with NTFF tracing; returns exec time in ns (or None)."""
    _ensure_ntff_hook()
    w = _prep_weights(inputs)
    w["ident"] = np.eye(C, dtype=np.float16)
    wspecs = _weight_specs(w)
    nc = _get_nc("v1", wspecs)
    x1 = np.ascontiguousarray(inputs["x1"], np.float32).reshape(B, C, N)
    x2 = np.ascontiguousarray(inputs["x2"], np.float32).reshape(B, C, N)
    in_maps = [dict(w, x1=x1[b], x2=x2[b]) for b in range(B)]
    res = run_bass_kernel_spmd(nc, in_maps, list(range(NCORES)), trace=True)
    globals()["_LAST_TRACE"] = res
    return res.exec_time_ns

